# revision 5
# baseline (speedup 1.0000x reference)
"""Single-query masked attention (sparse_attention) for Trainium2, 8 NeuronCores.

Math (per batch b):
    k = enc @ Wk + bk ; e[t] = (q . k[t]) / sqrt(Dk)
    attn = softmax(e masked to t < len) ; out = sum_t attn[t] * (enc[t] @ Wv + bv)

Reformulated so enc (the only big tensor, 268 MB) is read exactly once:
    qt = (Wk @ q) / sqrt(Dk)        (host, tiny)   -> e[t] = enc[t] . qt  (+const, cancels)
    p[t] = exp(e[t]) * mask01[t]    ;  s = sum p
    pooled = sum_t p[t] * enc[t]    (PE matmul, accumulated in PSUM)
    out = (pooled @ Wv) / s + bv    (tiny matmuls on-chip)

Sharding: data-parallel over batch, 4 batches per core, 8 cores.
Per tile of 128 tokens x 512 features:
  - energies: one DVE tensor_tensor_reduce (fused multiply + free-dim reduce)
  - pooled:   one PE matmul per tile (weights = the 128 token probs, 1 column)
"""

import sys

sys.path.insert(0, "/opt/trn_rl_repo")

import numpy as np

B, T, D_IN, D_K, D_V = 32, 4096, 512, 128, 128
N_CORES = 8
B_LOC = B // N_CORES          # 4 batches per core
P = 128                       # partition tile of tokens
N_TILES = T // P              # 32 token tiles per batch
TILES_PER_DMA = 4             # 1 MiB per enc DMA
N_DMAS = N_TILES // TILES_PER_DMA
D_CHUNKS = D_IN // 128        # 4

_cache = {}


def _build_program():
    from concourse import bacc, mybir, tile
    import concourse.bass as bass

    f32 = mybir.dt.float32
    Alu = mybir.AluOpType
    Act = mybir.ActivationFunctionType

    nc = bacc.Bacc(
        "TRN2",
        target_bir_lowering=False,
        debug=False,
        enable_asserts=False,
        num_devices=N_CORES,
    )

    enc_d = nc.dram_tensor("enc", [B_LOC, T, D_IN], f32, kind="ExternalInput").ap()
    qt_d = nc.dram_tensor("qt", [B_LOC, P, D_IN], f32, kind="ExternalInput").ap()
    mask_d = nc.dram_tensor("mask", [B_LOC, P, N_TILES], f32, kind="ExternalInput").ap()
    wv_d = nc.dram_tensor("wv", [D_IN, D_V], f32, kind="ExternalInput").ap()
    bv_d = nc.dram_tensor("bv", [D_V, 1], f32, kind="ExternalInput").ap()
    out_d = nc.dram_tensor("out", [B_LOC, D_V, 1], f32, kind="ExternalOutput").ap()

    with tile.TileContext(nc) as tc:
        with (
            tc.tile_pool(name="enc", bufs=2 * N_DMAS) as enc_pool,
            tc.tile_pool(name="qt", bufs=2) as qt_pool,
            tc.tile_pool(name="tmp", bufs=4) as tmp_pool,
            tc.tile_pool(name="small", bufs=4) as small_pool,
            tc.tile_pool(name="const", bufs=1) as const_pool,
            tc.tile_pool(name="psum_big", bufs=2, space="PSUM") as psum_big,
            tc.tile_pool(name="psum_sm", bufs=4, space="PSUM") as psum_sm,
        ):
            # constants
            ones_sb = const_pool.tile([P, P], f32, tag="ones")
            nc.vector.memset(ones_sb[:], 1.0)
            one_sb = const_pool.tile([1, 1], f32, tag="one")
            nc.vector.memset(one_sb[:], 1.0)
            wv_sb = const_pool.tile([P, D_IN], f32, tag="wv")
            for dc in range(D_CHUNKS):
                nc.sync.dma_start(
                    out=wv_sb[:, dc * 128 : (dc + 1) * 128],
                    in_=wv_d[dc * 128 : (dc + 1) * 128, :],
                )
            bv_sb = const_pool.tile([D_V, 1], f32, tag="bv")
            nc.sync.dma_start(out=bv_sb[:], in_=bv_d[:])

            for b in range(B_LOC):
                qt_sb = qt_pool.tile([P, D_IN], f32, tag="qt")
                nc.sync.dma_start(out=qt_sb[:], in_=qt_d[b])
                mask_sb = small_pool.tile([P, N_TILES], f32, tag="mask")
                nc.sync.dma_start(out=mask_sb[:], in_=mask_d[b])

                enc_tiles = []
                for j in range(N_DMAS):
                    et = enc_pool.tile([P, TILES_PER_DMA, D_IN], f32, tag="enc")
                    src = enc_d[b, j * P * TILES_PER_DMA : (j + 1) * P * TILES_PER_DMA, :]
                    nc.sync.dma_start(
                        out=et[:], in_=src.rearrange("(i p) d -> p i d", p=P)
                    )
                    enc_tiles.append(et)

                # energies: e[p, i] = enc[i*128+p, :] . qt
                # DVE multiply, then ACT free-dim reduce via accum_out.
                e_sb = small_pool.tile([P, N_TILES], f32, tag="e")
                for i in range(N_TILES):
                    j, il = divmod(i, TILES_PER_DMA)
                    tmp = tmp_pool.tile([P, D_IN], f32, tag="tmp")
                    nc.vector.tensor_tensor(
                        out=tmp[:],
                        in0=enc_tiles[j][:, il, :],
                        in1=qt_sb[:],
                        op=Alu.mult,
                    )
                    nc.scalar.activation(
                        tmp[:], tmp[:], Act.Copy, accum_out=e_sb[:, i : i + 1]
                    )

                # additive mask (0 or -1e5), then p = exp(e), srow = row-sums of p
                # (no max shift needed: |e| is O(1) for this problem)
                nc.vector.tensor_tensor(
                    out=e_sb[:], in0=e_sb[:], in1=mask_sb[:], op=Alu.add
                )
                pm_sb = small_pool.tile([P, N_TILES], f32, tag="pm")
                srow_sb = small_pool.tile([P, 1], f32, tag="srow")
                nc.scalar.activation(pm_sb[:], e_sb[:], Act.Exp, accum_out=srow_sb[:])
                s_ps = psum_sm.tile([P, 1], f32, tag="sm")
                nc.tensor.matmul(s_ps[:], ones_sb[:], srow_sb[:], start=True, stop=True)
                rs_sb = small_pool.tile([P, 1], f32, tag="rs")
                nc.vector.reciprocal(rs_sb[:], s_ps[:])

                # pooled[1, 512] += pm[:, i].T @ enc_tile_i
                pool_ps = psum_big.tile([1, D_IN], f32, tag="pooled")
                for i in range(N_TILES):
                    j, il = divmod(i, TILES_PER_DMA)
                    nc.tensor.matmul(
                        pool_ps[:],
                        pm_sb[:, i : i + 1],
                        enc_tiles[j][:, il, :],
                        start=(i == 0),
                        stop=(i == N_TILES - 1),
                    )
                pooled_sb = small_pool.tile([1, D_IN], f32, tag="pooled_sb")
                nc.vector.tensor_copy(pooled_sb[:], pool_ps[:])

                # transpose pooled [1,512] -> [128,4] via K=1 matmuls
                poolT_ps = psum_sm.tile([P, D_CHUNKS], f32, tag="sm")
                for dc in range(D_CHUNKS):
                    nc.tensor.matmul(
                        poolT_ps[:, dc : dc + 1],
                        pooled_sb[0:1, dc * 128 : (dc + 1) * 128],
                        one_sb[:],
                        start=True,
                        stop=True,
                    )
                poolT_sb = small_pool.tile([P, D_CHUNKS], f32, tag="poolT_sb")
                nc.vector.tensor_copy(poolT_sb[:], poolT_ps[:])

                # context[v] = sum_d Wv[d, v] * pooled[d]
                ctx_ps = psum_sm.tile([D_V, 1], f32, tag="sm")
                for dc in range(D_CHUNKS):
                    nc.tensor.matmul(
                        ctx_ps[:],
                        wv_sb[:, dc * 128 : (dc + 1) * 128],
                        poolT_sb[:, dc : dc + 1],
                        start=(dc == 0),
                        stop=(dc == D_CHUNKS - 1),
                    )
                ctx_sb = small_pool.tile([D_V, 1], f32, tag="ctx")
                nc.vector.tensor_scalar(
                    out=ctx_sb[:],
                    in0=ctx_ps[:],
                    scalar1=rs_sb[:],
                    scalar2=bv_sb[:],
                    op0=Alu.mult,
                    op1=Alu.add,
                )
                nc.sync.dma_start(out=out_d[b], in_=ctx_sb[:])

    nc.compile()
    return nc


def _get_program():
    if "nc" not in _cache:
        _cache["nc"] = _build_program()
    return _cache["nc"]


def _host_prep(enc_output, query, factorized_data_lens, Wk, bk, Wv, bv):
    """Build per-core input maps (host-side sharding + tiny folds)."""
    enc = np.ascontiguousarray(enc_output, dtype=np.float32)
    q = np.asarray(query, dtype=np.float32)
    Wk = np.asarray(Wk, dtype=np.float32)
    Wv = np.ascontiguousarray(Wv, dtype=np.float32)
    bv = np.asarray(bv, dtype=np.float32)
    lens = np.asarray(factorized_data_lens).astype(np.int64)

    scale = 1.0 / np.sqrt(np.float32(D_K))
    qt = (q @ Wk.T) * scale                             # [B, D_IN]
    qt_rep = np.ascontiguousarray(
        np.broadcast_to(qt[:, None, :], (B, P, D_IN)), dtype=np.float32
    )
    t_idx = np.arange(T, dtype=np.int64)
    valid = t_idx[None, :] < lens[:, None]                      # [B, T]
    madd = np.where(valid, np.float32(0.0), np.float32(-1e5)).astype(np.float32)
    mask = np.ascontiguousarray(madd.reshape(B, N_TILES, P).transpose(0, 2, 1))
    bv2 = np.ascontiguousarray(bv.reshape(D_V, 1))

    in_maps = []
    for c in range(N_CORES):
        sl = slice(c * B_LOC, (c + 1) * B_LOC)
        in_maps.append(
            {
                "enc": enc[sl],
                "qt": qt_rep[sl],
                "mask": mask[sl],
                "wv": Wv,
                "bv": bv2,
            }
        )
    return in_maps


def run(inputs, trace=False, trace_cores=None):
    """Run on 8 cores; returns (output [B, D_V] f32, BassKernelResults)."""
    from concourse.bass_utils import run_bass_kernel_spmd

    nc = _get_program()
    in_maps = _host_prep(**inputs)
    res = run_bass_kernel_spmd(
        nc,
        in_maps,
        list(range(N_CORES)),
        trace=trace,
        trace_cores=trace_cores,
    )
    out = np.concatenate(
        [res.results[c]["out"].reshape(B_LOC, D_V) for c in range(N_CORES)], axis=0
    )
    return out, res


def kernel(**inputs) -> np.ndarray:
    out, _ = run(inputs, trace=False)
    return out


def make_bench(inputs):
    """Build a reusable jitted 8-core executable + device-resident inputs.

    Returns (step_fn, check_fn): step_fn() queues one execution and returns
    the output jax arrays; caller blocks when desired.
    """
    import jax
    import numpy as _np
    from jax.experimental.shard_map import shard_map
    from jax.sharding import Mesh, PartitionSpec

    from concourse import bass2jax, mybir

    bass2jax.install_neuronx_cc_hook()
    nc = _get_program()
    in_maps = _host_prep(**inputs)
    n_cores = N_CORES

    partition_name = nc.partition_id_tensor.name if nc.partition_id_tensor else None
    in_names, out_names, out_avals = [], [], []
    for alloc in nc.m.functions[0].allocations:
        if not isinstance(alloc, mybir.MemoryLocationSet):
            continue
        name = alloc.memorylocations[0].name
        if alloc.kind == "ExternalInput":
            if name != partition_name:
                in_names.append(name)
        elif alloc.kind == "ExternalOutput":
            out_names.append(name)
            out_avals.append(
                jax.core.ShapedArray(tuple(alloc.tensor_shape), mybir.dt.np(alloc.dtype))
            )
    n_params = len(in_names)
    all_names = in_names + out_names
    if partition_name is not None:
        all_names = all_names + [partition_name]

    def _body(*args):
        operands = list(args)
        if partition_name is not None:
            operands.append(bass2jax.partition_id_tensor())
        outs = bass2jax._bass_exec_p.bind(
            *operands,
            out_avals=tuple(out_avals),
            in_names=tuple(all_names),
            out_names=tuple(out_names),
            lowering_input_output_aliases=(),
            sim_require_finite=True,
            sim_require_nnan=True,
            nc=nc,
        )
        return tuple(outs)

    devices = jax.devices()[:n_cores]
    mesh = Mesh(_np.asarray(devices), ("core",))
    n_outs = len(out_names)
    sharded = jax.jit(
        shard_map(
            _body,
            mesh=mesh,
            in_specs=(PartitionSpec("core"),) * (n_params + n_outs),
            out_specs=(PartitionSpec("core"),) * n_outs,
            check_rep=False,
        ),
        keep_unused=True,
    )

    sh = jax.sharding.NamedSharding(mesh, PartitionSpec("core"))
    concat_in = [
        jax.device_put(
            _np.concatenate([_np.asarray(in_maps[c][n]) for c in range(n_cores)], axis=0),
            sh,
        )
        for n in in_names
    ]
    concat_zero = [
        jax.device_put(
            _np.zeros((n_cores * a.shape[0], *a.shape[1:]), a.dtype), sh
        )
        for a in out_avals
    ]

    def step():
        return sharded(*concat_in, *concat_zero)

    def gather(outs):
        o = _np.asarray(outs[0]).reshape(n_cores, B_LOC, D_V)
        return o.reshape(B, D_V)

    return step, gather


# revision 12
# speedup vs baseline: 11.8527x; 11.8527x over previous
"""Single-query masked attention (sparse_attention) for Trainium2, 8 NeuronCores.

Math (per batch b):
    k = enc @ Wk + bk ; e[t] = (q . k[t]) / sqrt(Dk)
    attn = softmax(e masked to t < len) ; out = sum_t attn[t] * (enc[t] @ Wv + bv)

Reformulated so enc (the only big tensor, 268 MB) is read exactly once:
    qt = (Wk @ q) / sqrt(Dk)        (host, tiny)   -> e[t] = enc[t] . qt  (+const, cancels)
    p[t] = exp(e[t]) * mask01[t]    ;  s = sum p
    pooled = sum_t p[t] * enc[t]    (PE matmul, accumulated in PSUM)
    out = (pooled @ Wv) / s + bv    (tiny matmuls on-chip)

Sharding: data-parallel over batch, 4 batches per core, 8 cores.
Per tile of 128 tokens x 512 features:
  - energies: one DVE tensor_tensor_reduce (fused multiply + free-dim reduce)
  - pooled:   one PE matmul per tile (weights = the 128 token probs, 1 column)
"""

import sys

sys.path.insert(0, "/opt/trn_rl_repo")

import numpy as np

B, T, D_IN, D_K, D_V = 32, 4096, 512, 128, 128
N_CORES = 8
B_LOC = B // N_CORES          # 4 batches per core
P = 128                       # partition tile of tokens
N_TILES = T // P              # 32 token tiles per batch
TILES_PER_DMA = 4             # 1 MiB per enc DMA
N_DMAS = N_TILES // TILES_PER_DMA
D_CHUNKS = D_IN // 128        # 4

_cache = {}


def _build_program(repeat=1):
    from concourse import bacc, mybir, tile
    import concourse.bass as bass

    f32 = mybir.dt.float32
    Alu = mybir.AluOpType
    Act = mybir.ActivationFunctionType

    nc = bacc.Bacc(
        "TRN2",
        target_bir_lowering=False,
        debug=False,
        enable_asserts=False,
        num_devices=N_CORES,
    )

    enc_d = nc.dram_tensor("enc", [B_LOC, T, D_IN], f32, kind="ExternalInput").ap()
    qt_d = nc.dram_tensor("qt", [B_LOC, P, D_IN], f32, kind="ExternalInput").ap()
    mask_d = nc.dram_tensor("mask", [B_LOC, P, N_TILES], f32, kind="ExternalInput").ap()
    wv_d = nc.dram_tensor("wv", [D_IN, D_V], f32, kind="ExternalInput").ap()
    bv_d = nc.dram_tensor("bv", [D_V, 1], f32, kind="ExternalInput").ap()
    out_d = nc.dram_tensor("out", [B_LOC, D_V, 1], f32, kind="ExternalOutput").ap()

    with tile.TileContext(nc) as tc:
        with (
            tc.tile_pool(name="enc", bufs=2 * N_DMAS) as enc_pool,
            tc.tile_pool(name="qt", bufs=2) as qt_pool,
            tc.tile_pool(name="tmp", bufs=4) as tmp_pool,
            tc.tile_pool(name="small", bufs=4) as small_pool,
            tc.tile_pool(name="const", bufs=1) as const_pool,
            tc.tile_pool(name="psum_big", bufs=2, space="PSUM") as psum_big,
            tc.tile_pool(name="psum_sm", bufs=4, space="PSUM") as psum_sm,
        ):
            # constants
            ones_sb = const_pool.tile([P, P], f32, tag="ones")
            nc.vector.memset(ones_sb[:], 1.0)
            one_sb = const_pool.tile([1, 1], f32, tag="one")
            nc.vector.memset(one_sb[:], 1.0)
            wv_sb = const_pool.tile([P, D_IN], f32, tag="wv")
            for dc in range(D_CHUNKS):
                nc.sync.dma_start(
                    out=wv_sb[:, dc * 128 : (dc + 1) * 128],
                    in_=wv_d[dc * 128 : (dc + 1) * 128, :],
                )
            bv_sb = const_pool.tile([D_V, 1], f32, tag="bv")
            nc.sync.dma_start(out=bv_sb[:], in_=bv_d[:])

            for b in [b for _ in range(repeat) for b in range(B_LOC)]:
                qt_sb = qt_pool.tile([P, D_IN], f32, tag="qt")
                nc.sync.dma_start(out=qt_sb[:], in_=qt_d[b])
                mask_sb = small_pool.tile([P, N_TILES], f32, tag="mask")
                nc.sync.dma_start(out=mask_sb[:], in_=mask_d[b])

                enc_tiles = []
                for j in range(N_DMAS):
                    et = enc_pool.tile([P, TILES_PER_DMA, D_IN], f32, tag="enc")
                    src = enc_d[b, j * P * TILES_PER_DMA : (j + 1) * P * TILES_PER_DMA, :]
                    nc.sync.dma_start(
                        out=et[:], in_=src.rearrange("(i p) d -> p i d", p=P)
                    )
                    enc_tiles.append(et)

                # energies: e[p, i] = enc[i*128+p, :] . qt
                # DVE multiply, then ACT free-dim reduce via accum_out.
                e_sb = small_pool.tile([P, N_TILES], f32, tag="e")
                for i in range(N_TILES):
                    j, il = divmod(i, TILES_PER_DMA)
                    tmp = tmp_pool.tile([P, D_IN], f32, tag="tmp")
                    nc.vector.tensor_tensor(
                        out=tmp[:],
                        in0=enc_tiles[j][:, il, :],
                        in1=qt_sb[:],
                        op=Alu.mult,
                    )
                    nc.scalar.activation(
                        tmp[:], tmp[:], Act.Copy, accum_out=e_sb[:, i : i + 1]
                    )

                # additive mask (0 or -1e5), then p = exp(e), srow = row-sums of p
                # (no max shift needed: |e| is O(1) for this problem)
                nc.vector.tensor_tensor(
                    out=e_sb[:], in0=e_sb[:], in1=mask_sb[:], op=Alu.add
                )
                pm_sb = small_pool.tile([P, N_TILES], f32, tag="pm")
                srow_sb = small_pool.tile([P, 1], f32, tag="srow")
                nc.scalar.activation(pm_sb[:], e_sb[:], Act.Exp, accum_out=srow_sb[:])
                s_ps = psum_sm.tile([P, 1], f32, tag="sm")
                nc.tensor.matmul(s_ps[:], ones_sb[:], srow_sb[:], start=True, stop=True)
                rs_sb = small_pool.tile([P, 1], f32, tag="rs")
                nc.vector.reciprocal(rs_sb[:], s_ps[:])

                # pooled[1, 512] += pm[:, i].T @ enc_tile_i
                pool_ps = psum_big.tile([1, D_IN], f32, tag="pooled")
                for i in range(N_TILES):
                    j, il = divmod(i, TILES_PER_DMA)
                    nc.tensor.matmul(
                        pool_ps[:],
                        pm_sb[:, i : i + 1],
                        enc_tiles[j][:, il, :],
                        start=(i == 0),
                        stop=(i == N_TILES - 1),
                    )
                pooled_sb = small_pool.tile([1, D_IN], f32, tag="pooled_sb")
                nc.vector.tensor_copy(pooled_sb[:], pool_ps[:])

                # transpose pooled [1,512] -> [128,4] via K=1 matmuls
                poolT_ps = psum_sm.tile([P, D_CHUNKS], f32, tag="sm")
                for dc in range(D_CHUNKS):
                    nc.tensor.matmul(
                        poolT_ps[:, dc : dc + 1],
                        pooled_sb[0:1, dc * 128 : (dc + 1) * 128],
                        one_sb[:],
                        start=True,
                        stop=True,
                    )
                poolT_sb = small_pool.tile([P, D_CHUNKS], f32, tag="poolT_sb")
                nc.vector.tensor_copy(poolT_sb[:], poolT_ps[:])

                # context[v] = sum_d Wv[d, v] * pooled[d]
                ctx_ps = psum_sm.tile([D_V, 1], f32, tag="sm")
                for dc in range(D_CHUNKS):
                    nc.tensor.matmul(
                        ctx_ps[:],
                        wv_sb[:, dc * 128 : (dc + 1) * 128],
                        poolT_sb[:, dc : dc + 1],
                        start=(dc == 0),
                        stop=(dc == D_CHUNKS - 1),
                    )
                ctx_sb = small_pool.tile([D_V, 1], f32, tag="ctx")
                nc.vector.tensor_scalar(
                    out=ctx_sb[:],
                    in0=ctx_ps[:],
                    scalar1=rs_sb[:],
                    scalar2=bv_sb[:],
                    op0=Alu.mult,
                    op1=Alu.add,
                )
                nc.sync.dma_start(out=out_d[b], in_=ctx_sb[:])

    nc.compile()
    return nc


def _get_program(repeat=1):
    key = f"nc{repeat}"
    if key not in _cache:
        _cache[key] = _build_program(repeat)
    return _cache[key]


def _host_prep(enc_output, query, factorized_data_lens, Wk, bk, Wv, bv):
    """Build per-core input maps (host-side sharding + tiny folds)."""
    enc = np.ascontiguousarray(enc_output, dtype=np.float32)
    q = np.asarray(query, dtype=np.float32)
    Wk = np.asarray(Wk, dtype=np.float32)
    Wv = np.ascontiguousarray(Wv, dtype=np.float32)
    bv = np.asarray(bv, dtype=np.float32)
    lens = np.asarray(factorized_data_lens).astype(np.int64)

    scale = 1.0 / np.sqrt(np.float32(D_K))
    qt = (q @ Wk.T) * scale                             # [B, D_IN]
    qt_rep = np.ascontiguousarray(
        np.broadcast_to(qt[:, None, :], (B, P, D_IN)), dtype=np.float32
    )
    t_idx = np.arange(T, dtype=np.int64)
    valid = t_idx[None, :] < lens[:, None]                      # [B, T]
    madd = np.where(valid, np.float32(0.0), np.float32(-1e5)).astype(np.float32)
    mask = np.ascontiguousarray(madd.reshape(B, N_TILES, P).transpose(0, 2, 1))
    bv2 = np.ascontiguousarray(bv.reshape(D_V, 1))

    in_maps = []
    for c in range(N_CORES):
        sl = slice(c * B_LOC, (c + 1) * B_LOC)
        in_maps.append(
            {
                "enc": enc[sl],
                "qt": qt_rep[sl],
                "mask": mask[sl],
                "wv": Wv,
                "bv": bv2,
            }
        )
    return in_maps


def run(inputs, trace=False, trace_cores=None):
    """Run on 8 cores; returns (output [B, D_V] f32, BassKernelResults)."""
    from concourse.bass_utils import run_bass_kernel_spmd

    nc = _get_program()
    in_maps = _host_prep(**inputs)
    res = run_bass_kernel_spmd(
        nc,
        in_maps,
        list(range(N_CORES)),
        trace=trace,
        trace_cores=trace_cores,
    )
    out = np.concatenate(
        [res.results[c]["out"].reshape(B_LOC, D_V) for c in range(N_CORES)], axis=0
    )
    return out, res


def kernel(**inputs) -> np.ndarray:
    out, _ = run(inputs, trace=False)
    return out


def make_bench(inputs, chain=1, repeat=1):
    """Build a reusable jitted 8-core executable + device-resident inputs.

    `chain` = number of back-to-back NEFF executions inside one jitted call
    (output buffer threaded through as a data dependency).
    Returns (step_fn, check_fn): step_fn() queues one call and returns
    the output jax arrays; caller blocks when desired.
    """
    import jax
    import numpy as _np
    from jax.experimental.shard_map import shard_map
    from jax.sharding import Mesh, PartitionSpec

    from concourse import bass2jax, mybir

    bass2jax.install_neuronx_cc_hook()
    nc = _get_program(repeat)
    in_maps = _host_prep(**inputs)
    n_cores = N_CORES

    partition_name = nc.partition_id_tensor.name if nc.partition_id_tensor else None
    in_names, out_names, out_avals = [], [], []
    for alloc in nc.m.functions[0].allocations:
        if not isinstance(alloc, mybir.MemoryLocationSet):
            continue
        name = alloc.memorylocations[0].name
        if alloc.kind == "ExternalInput":
            if name != partition_name:
                in_names.append(name)
        elif alloc.kind == "ExternalOutput":
            out_names.append(name)
            out_avals.append(
                jax.core.ShapedArray(tuple(alloc.tensor_shape), mybir.dt.np(alloc.dtype))
            )
    n_params = len(in_names)
    all_names = in_names + out_names
    if partition_name is not None:
        all_names = all_names + [partition_name]

    def _body(*args):
        ins = list(args[:n_params])
        outs = list(args[n_params:])
        pid = bass2jax.partition_id_tensor() if partition_name is not None else None
        for _ in range(chain):
            operands = ins + outs
            if pid is not None:
                operands = operands + [pid]
            outs = list(
                bass2jax._bass_exec_p.bind(
                    *operands,
                    out_avals=tuple(out_avals),
                    in_names=tuple(all_names),
                    out_names=tuple(out_names),
                    lowering_input_output_aliases=(),
                    sim_require_finite=True,
                    sim_require_nnan=True,
                    nc=nc,
                )
            )
        return tuple(outs)

    devices = jax.devices()[:n_cores]
    mesh = Mesh(_np.asarray(devices), ("core",))
    n_outs = len(out_names)
    sharded = jax.jit(
        shard_map(
            _body,
            mesh=mesh,
            in_specs=(PartitionSpec("core"),) * (n_params + n_outs),
            out_specs=(PartitionSpec("core"),) * n_outs,
            check_rep=False,
        ),
        keep_unused=True,
    )

    sh = jax.sharding.NamedSharding(mesh, PartitionSpec("core"))
    concat_in = [
        jax.device_put(
            _np.concatenate([_np.asarray(in_maps[c][n]) for c in range(n_cores)], axis=0),
            sh,
        )
        for n in in_names
    ]
    concat_zero = [
        jax.device_put(
            _np.zeros((n_cores * a.shape[0], *a.shape[1:]), a.dtype), sh
        )
        for a in out_avals
    ]

    def step():
        return sharded(*concat_in, *concat_zero)

    def gather(outs):
        o = _np.asarray(outs[0]).reshape(n_cores, B_LOC, D_V)
        return o.reshape(B, D_V)

    return step, gather
